# revision 10
# baseline (speedup 1.0000x reference)
"""Trainium2 Bass kernel for a BasicTransformerBlock (self-attn + cross-attn + GEGLU FF).

Contract: kernel(**inputs) takes FULL unsharded inputs (np arrays keyed as in
setup_inputs()) and returns the FULL [8, 1024, 512] float32 output.

Sharding: data-parallel over batch B=8 across 8 NeuronCores (one batch element
per core, all weights replicated, no collectives).

Structure (per core, tokens L=1024, model D=512, heads H=8 x DH=64):
  - LayerNorm token-major (bn_stats), z in bf16, PE-transposed per channel
    chunk into one packed PSUM bank, single 2x-mode DVE eviction per chunk
    applying gamma/beta -> feature-major hT [128ch, 1024tok] bf16.
  - Projections feature-major: weights stationary (fp32r), hT moving (bf16).
  - Self-attention: per head-pair hp, scores S^T = K Q^T into a 2-bank PSUM
    tile (both 64-part head halves side by side), ONE exp -> bf16; AV rides
    the softmax denominator as V's extra ones-column; normalization via DVE
    reciprocal + GpSimd partition_broadcast + DVE multiply (no PE, no ACT).
  - Cross-attention analogous (Lk = 77).
  - GEGLU FF: ff_w1 streamed from HBM (read once), val+gate evictions both on
    ACT (bias fused), product on DVE in 4x mode (bf16, SBUF-only), ff2
    accumulated with rank-1 bias preload, residual add, DMA out.
  - attn scale DH^-0.5 is folded into a1_wq/a2_wq on the host.
"""

import os

import numpy as np

import concourse.bass as bass
import concourse.tile as tile
from concourse import mybir
from concourse.bass_utils import run_bass_kernel_spmd
from concourse.masks import make_identity

# ---------------------------------------------------------------------------
# Workaround: this toolchain's walrus encodes at most ONE sync-wait per
# instruction (2 for EventSemaphore). Tile attaches one wait per producer
# proc, so after scheduling we hoist excess waits onto prepended same-engine
# NOPs -- semantically identical (the engine blocks at the NOP instead).
# ---------------------------------------------------------------------------
def _legalize_wait_counts(nc, max_waits=1):
    n_moved = 0
    for f in nc.m.functions:
        for bb in f.blocks:
            out, changed = [], False
            for inst in bb.instructions:
                si = inst.sync_info
                waits = list(si.on_wait) if si is not None and si.on_wait else []
                cap = 2 if isinstance(inst, mybir.InstEventSemaphore) else max_waits
                if len(waits) > cap:
                    keep, rest = waits[:cap], waits[cap:]
                    for i in range(0, len(rest), max_waits):
                        out.append(mybir.InstNoOp(
                            name=f"{inst.name}-lw{i}",
                            engine=inst.engine,
                            bass_nofuse=True,
                            sync_info=mybir.SyncInfo(
                                on_wait=rest[i:i + max_waits], on_update=[]),
                        ))
                    si.on_wait = keep
                    inst.sync_info = si
                    n_moved += len(rest)
                    changed = True
                out.append(inst)
            if changed:
                bb.instructions = out
    return n_moved


# ---------------------------------------------------------------------------
# Problem shapes (hardcoded per contract)
# ---------------------------------------------------------------------------
P = 128
B, L, D, S, CD, H, DH = 8, 1024, 512, 77, 768, 8, 64
FF = 2048                 # GEGLU inner dim; ff_w1 is [D, 2*FF]
LT = L // P               # 8 token tiles
KC = D // P               # 4 channel chunks of the model dim
CC = CD // P              # 6 context channel chunks
NH = 512                  # moving-operand chunk (PSUM bank = 512 fp32)
TH = L // NH              # 2 token halves
FC = FF // P              # 16 chunks of the FF inner dim
EPS = 1e-5
NCORES = 8

F32 = mybir.dt.float32
F32R = mybir.dt.float32r
BF16 = mybir.dt.bfloat16
FP8 = mybir.dt.float8e4
AF = mybir.ActivationFunctionType
ALU = mybir.AluOpType


def _build_nc():
    nc = bass.Bass(target_bir_lowering=False, debug=(os.environ.get('BASS_KERNEL_DEBUG','0')=='1'))

    pr = {}
    pr["x"] = nc.declare_dram_parameter("x", [L, D], F32, isOutput=False)
    pr["context"] = nc.declare_dram_parameter("context", [S, CD], F32, isOutput=False)
    for nm in ("ln1_g", "ln1_b", "ln2_g", "ln2_b", "ln3_g", "ln3_b"):
        pr[nm] = nc.declare_dram_parameter(nm, [D], F32, isOutput=False)
    for nm in ("a1_bo", "a2_bo", "ff_b2"):
        # consumed (only) as fp32r matmul operands via the rank-1 bias trick
        pr[nm] = nc.declare_dram_parameter(nm, [D], F32R, isOutput=False)
    for nm in ("a1_wq", "a1_wk", "a1_wv", "a1_wo", "a2_wq", "a2_wo"):
        pr[nm] = nc.declare_dram_parameter(nm, [D, D], F32, isOutput=False)
    for nm in ("a2_wk", "a2_wv"):
        pr[nm] = nc.declare_dram_parameter(nm, [CD, D], F32, isOutput=False)
    pr["ff_w1"] = nc.declare_dram_parameter("ff_w1", [D, 2 * FF], F32, isOutput=False)
    pr["ff_b1"] = nc.declare_dram_parameter("ff_b1", [2 * FF], F32, isOutput=False)
    pr["ff_w2"] = nc.declare_dram_parameter("ff_w2", [FF, D], F32, isOutput=False)
    out_p = nc.declare_dram_parameter("out", [L, D], F32, isOutput=True)

    reps = int(os.environ.get("BASS_KERNEL_REPS", "1"))
    with tile.TileContext(nc) as tc, \
         nc.allow_low_precision(reason="bf16/fp32r tiles feed matmuls; all "
                                       "matmul accumulation stays fp32 in PSUM"):
        if reps > 1:
            with tc.For_i(0, reps, 1):
                _emit(nc, tc, pr, out_p)
        else:
            _emit(nc, tc, pr, out_p)
    if os.environ.get("BASS_KERNEL_SKIP_WAIT_LEGALIZE") != "1":
        _legalize_wait_counts(nc)
    return nc


def _emit(nc, tc, pr, out_p):
    from contextlib import ExitStack

    top = ExitStack()
    with top:
        # ------------------ persistent pools (whole kernel) -----------------
        const = top.enter_context(tc.tile_pool(name="const", bufs=1))
        xpool = top.enter_context(tc.tile_pool(name="xsb", bufs=1))
        htpool = top.enter_context(tc.tile_pool(name="hT", bufs=1))
        lntp = top.enter_context(tc.tile_pool(name="lnt", bufs=4))
        zpool = top.enter_context(tc.tile_pool(name="zp", bufs=1))
        qkpool = top.enter_context(tc.tile_pool(name="qkT", bufs=1))
        opool = top.enter_context(tc.tile_pool(name="oT", bufs=1))
        wstage = top.enter_context(tc.tile_pool(name="wstage", bufs=5))

        # ----------------------------- constants ---------------------------
        # x tiles first on the HWDGE queue; tiny constant DMAs go through the
        # GpSimd SWDGE queue so they don't serialize behind them
        lnstack = const.tile([P, P], F32, tag="lnstack", name="lnstack")
        for i, (gnm, bnm) in enumerate((("ln1_g", "ln1_b"), ("ln2_g", "ln2_b"),
                                        ("ln3_g", "ln3_b"))):
            nc.sync.dma_start(out=lnstack[i * 8:i * 8 + 4, :],
                                in_=pr[gnm].rearrange("(a f) -> a f", f=P))
            nc.sync.dma_start(out=lnstack[i * 8 + 4:i * 8 + 8, :],
                                in_=pr[bnm].rearrange("(a f) -> a f", f=P))
        fb1stack = const.tile([32, P], F32, tag="fb1stack", name="fb1stack")
        nc.sync.dma_start(out=fb1stack[:, :],
                            in_=pr["ff_b1"].rearrange("(a f) -> a f", f=P))
        # bias rows for the rank-1 PSUM-preload trick
        bo1_row = const.tile([1, D], F32R, tag="bo1row", name="bo1row")
        nc.sync.dma_start(out=bo1_row[:, :],
                            in_=pr["a1_bo"].rearrange("(o f) -> o f", o=1))
        bo2_row = const.tile([1, D], F32R, tag="bo2row", name="bo2row")
        nc.sync.dma_start(out=bo2_row[:, :],
                            in_=pr["a2_bo"].rearrange("(o f) -> o f", o=1))
        fb2_row = const.tile([1, D], F32R, tag="fb2row", name="fb2row")
        nc.sync.dma_start(out=fb2_row[:, :],
                            in_=pr["ff_b2"].rearrange("(o f) -> o f", o=1))

        identity = const.tile([P, P], F32, tag="identity", name="identity")
        make_identity(nc, identity[:, :])
        idbf = const.tile([P, P], BF16, tag="idbf", name="idbf")
        nc.gpsimd.tensor_copy(idbf[:, :], identity[:, :])
        ones_f = const.tile([1, P], F32, tag="ones_f", name="ones_f")
        nc.vector.memset(ones_f[:, :], 1.0)
        ones = const.tile([1, P], F32R, tag="ones", name="ones")
        nc.vector.tensor_copy(ones[:, :], ones_f[:, :])
        eps_t = const.tile([P, 1], F32, tag="eps", name="eps")
        nc.vector.memset(eps_t[:, :], EPS)
        neg5 = const.tile([P, 1], F32, tag="neg5", name="neg5")
        nc.vector.memset(neg5[:, :], -5.0)

        # ------------------------------ DMAs in -----------------------------
        xsb = []
        for t in range(LT):
            xt = xpool.tile([P, D], F32, tag=f"x{t}", name=f"x{t}")
            nc.sync.dma_start(out=xt[:, :], in_=pr["x"][t * P:(t + 1) * P, :])
            xsb.append(xt)
        ctx = xpool.tile([P, CD], F32, tag="ctx", name="ctx")
        nc.sync.dma_start(out=ctx[0:S, :], in_=pr["context"][:, :])

        def load_w_pk(pool, nm, tag):
            """[512,512] weight -> packed fp8 [P, KC, D] (DoubleRow layout)."""
            w = pool.tile([P, KC, D], FP8, tag=tag, name=tag)
            for c in range(KC):
                wf = wstage.tile([P, D], F32, tag="stage", name=f"stg_{nm}{c}")
                nc.sync.dma_start(out=wf[:, :], in_=pr[nm][c * P:(c + 1) * P, :])
                nc.gpsimd.tensor_copy(w[:, c, :], wf[:, :])
            return w

        def load_w(pool, nm, rows, dst_dtype=F32, stage=None):
            """Load a [rows, 512] weight as row-chunk tiles; optional bf16
            cast staged through `stage` pool on the (otherwise idle) GpSimd."""
            tiles = []
            for c in range(rows // P):
                if stage is None:
                    w = pool.tile([P, D], dst_dtype, tag=f"{nm}{c}", name=f"{nm}{c}")
                    nc.sync.dma_start(out=w[:, :], in_=pr[nm][c * P:(c + 1) * P, :])
                else:
                    wf = stage.tile([P, D], F32, tag="stage", name=f"stg_{nm}{c}")
                    nc.sync.dma_start(out=wf[:, :], in_=pr[nm][c * P:(c + 1) * P, :])
                    w = pool.tile([P, D], dst_dtype, tag=f"{nm}{c}", name=f"{nm}{c}")
                    nc.gpsimd.tensor_copy(w[:, :], wf[:, :])
                tiles.append(w)
            return tiles

        es_a1 = ExitStack()
        a1pool = es_a1.enter_context(tc.tile_pool(name="a1w", bufs=1))
        wq1 = load_w_pk(a1pool, "a1_wq", "wq1pk")
        wk1 = load_w_pk(a1pool, "a1_wk", "wk1pk")
        wv1 = load_w_pk(a1pool, "a1_wv", "wv1pk")
        wo1 = load_w_pk(a1pool, "a1_wo", "wo1pk")

        # LN gamma/beta: six [512] vectors stacked as [4,128] rows -> [24,128],
        # one transpose -> gb [128,24]; ff_b1 [4096] -> [32,128] -> fb1 [128,32]
        gb = const.tile([P, 24], F32, tag="gb", name="gb")
        fb1 = const.tile([P, 32], F32, tag="fb1", name="fb1")
        with tc.tile_pool(name="psprep", bufs=2, space="PSUM") as psprep:
            pst = psprep.tile([P, P], F32, tag="pstr", name="pstr_gb")
            nc.tensor.transpose(pst[:, 0:24], lnstack[0:24, :], identity[0:24, 0:24])
            nc.vector.tensor_copy(gb[:, :], pst[:, 0:24])
            pst2 = psprep.tile([P, P], F32, tag="pstr", name="pstr_fb1")
            nc.tensor.transpose(pst2[:, 0:32], fb1stack[0:32, :], identity[0:32, 0:32])
            nc.vector.tensor_copy(fb1[:, :], pst2[:, 0:32])

        def g_col(i, kc):
            return gb[:, i * 8 + kc:i * 8 + kc + 1]

        def b_col(i, kc):
            return gb[:, i * 8 + 4 + kc:i * 8 + 4 + kc + 1]

        # ------------------------------------------------------------------
        # LayerNorm -> feature-major hT [128ch, 1024tok] bf16 x 4.
        # z in bf16; all 8 token-tile transposes of one channel chunk land in
        # ONE packed PSUM bank (bf16), evicted by a single 2x-mode DVE
        # tensor_scalar applying gamma/beta.
        # ------------------------------------------------------------------
        def ln_stats_tile(ln_i, t):
            """bn_stats -> rstd -> normalized z for one token tile."""
            st = lntp.tile([P, 6], F32, tag="bnstats", name="bnstats")
            nc.vector.bn_stats(st[:, :], xsb[t][:, :])
            mv = lntp.tile([P, 2], F32, tag="bnaggr", name="bnaggr")
            nc.vector.bn_aggr(mv[:, :], st[:, :])
            # rstd = exp(-0.5*ln(var+eps)); Ln/Exp share one ACT table
            # set with Identity/Copy (Sqrt would force a set switch).
            lnv = lntp.tile([P, 1], F32, tag="lnv", name="lnv")
            nc.scalar.activation(lnv[:, :], mv[:, 1:2], AF.Ln,
                                 bias=eps_t[:, :])
            rstd = lntp.tile([P, 1], F32, tag="rstd", name="rstd")
            nc.scalar.activation(rstd[:, :], lnv[:, :], AF.Exp, scale=-0.5)
            nmr = lntp.tile([P, 1], F32, tag="nmr", name="nmr")
            nc.vector.tensor_scalar(nmr[:, :], mv[:, 0:1], rstd[:, :], -1.0,
                                    op0=ALU.mult, op1=ALU.mult)
            z = zpool.tile([P, D], BF16, tag=f"z{t}", name=f"z{ln_i}_{t}")
            nc.scalar.activation(z[:, :], xsb[t][:, :], AF.Identity,
                                 bias=nmr[:, :], scale=rstd[:, :])
            return z

        def layer_norm_T(ln_i, zs=None):
            if zs is None:
                zs = [ln_stats_tile(ln_i, t) for t in range(LT)]
            h3 = htpool.tile([P, KC, L], FP8, tag="h3T8", name=f"hT8_{ln_i}")
            hT = [h3[:, c, :] for c in range(KC)]
            with tc.tile_pool(name=f"lnps{ln_i}", bufs=2, space="PSUM") as lnps:
                for c in range(KC):
                    ps = lnps.tile([P, L], BF16, tag="ps", name=f"lntr{c}")
                    for half in range(TH):
                        for t in range(half * LT // TH, (half + 1) * LT // TH):
                            nc.tensor.transpose(ps[:, t * P:(t + 1) * P],
                                                zs[t][:, c * P:(c + 1) * P],
                                                idbf[:, :])
                        sl = slice(half * NH, (half + 1) * NH)
                        # gamma/beta on ACT (idle here; DVE is the serial
                        # bottleneck in the LN windows)
                        nc.scalar.activation(hT[c][:, sl], ps[:, sl],
                                             AF.Identity,
                                             bias=b_col(ln_i, c),
                                             scale=g_col(ln_i, c))
            return h3

        # feature-major projection of one output chunk (both token halves into
        # a 2-bank PSUM tile, single eviction) -> [128, 1024] bf16
        DR = mybir.MatmulPerfMode.DoubleRow
        DESC = 1.0 / 64.0   # undo the host-side x64 fp8 range fold

        def proj_chunk(wpk, h8, pspool, oc, out_tag, pool=None, evict="dve"):
            pool = pool or qkpool
            ot = pool.tile([P, L], BF16, tag=out_tag, name=f"{out_tag}{oc}")
            for th in range(TH):
                ps = pspool.tile([P, NH], F32, tag="ps", name=f"ps_{out_tag}{oc}")
                for kk in range(KC // 2):
                    nc.tensor.matmul(
                        ps[:, :],
                        lhsT=wpk[:, 2 * kk:2 * kk + 2, oc * P:(oc + 1) * P],
                        rhs=h8[:, 2 * kk:2 * kk + 2, th * NH:(th + 1) * NH],
                        start=(kk == 0), stop=(kk == KC // 2 - 1),
                        perf_mode=DR)
                if evict == "act":
                    nc.scalar.mul(ot[:, th * NH:(th + 1) * NH], ps[:, :], DESC)
                else:
                    nc.vector.tensor_scalar(ot[:, th * NH:(th + 1) * NH],
                                            ps[:, :], DESC, None, op0=ALU.mult)
            return ot

        # out-projection + bias (rank-1 PSUM preload) + residual into xsb.
        # via_pool: stage PSUM->SBUF on ACT and add on GpSimd, freeing DVE in
        # the attention windows.
        def proj_tok_residual(o8, wpk, bias_row, nm, after_tile=None):
            with tc.tile_pool(name=f"psproj{nm}", bufs=3, space="PSUM") as psproj:
                for t in range(LT):
                    ps = psproj.tile([P, NH], F32, tag="ps", name="ps_proj")
                    nc.tensor.matmul(ps[:, :], lhsT=ones[0:1, 0:P],
                                     rhs=bias_row[:, :], start=True, stop=False)
                    for kk in range(KC // 2):
                        nc.tensor.matmul(
                            ps[:, :],
                            lhsT=o8[:, 2 * kk:2 * kk + 2, t * P:(t + 1) * P],
                            rhs=wpk[:, 2 * kk:2 * kk + 2, :],
                            start=False, stop=(kk == KC // 2 - 1),
                            perf_mode=DR)
                    nc.vector.scalar_tensor_tensor(
                        xsb[t][:, :], ps[:, :], DESC, xsb[t][:, :],
                        op0=ALU.mult, op1=ALU.add)
                    if after_tile is not None:
                        after_tile(t)

        # softmax normalization: denominator row (DH) of ps_o -> reciprocal
        # (DVE) -> rank-1 PE broadcast to 64 partitions -> evict -> multiply.
        # Emission is DEFERRED one unit by the callers so the PE ps_b matmul
        # never waits on the reciprocal.
        def normalize(ps_o, o_dst, attnsb, psb, evict="dve"):
            rec = attnsb.tile([1, NH], F32R, tag="rec", name="rec")
            nc.vector.reciprocal(rec[:, :], ps_o[DH:DH + 1, :])
            ps_b = psb.tile([P, NH], F32, tag="ps", name="ps_b")
            nc.tensor.matmul(ps_b[0:DH, :], lhsT=ones[0:1, 0:DH],
                             rhs=rec[:, :], start=True, stop=True)
            rb = attnsb.tile([DH, NH], F32, tag="rb", name="rb")
            if evict == "act":
                nc.scalar.copy(rb[:, :], ps_b[0:DH, :])
            else:
                nc.vector.tensor_copy(rb[:, :], ps_b[0:DH, :])
            nc.vector.tensor_mul(o_dst, ps_o[0:DH, :], rb[:, :])

        # ==================================================================
        # Section 1: self-attention
        # ==================================================================
        h1T = layer_norm_T(0)

        es_s1 = ExitStack()
        vpool = es_s1.enter_context(tc.tile_pool(name="vsb", bufs=1))
        o1T = opool.tile([P, KC, L], FP8, tag="o8", name="o1T8")

        with tc.tile_pool(name="psqkv", bufs=1, space="PSUM") as psqkv, \
             tc.tile_pool(name="qkt", bufs=2) as qktp, \
             tc.tile_pool(name="expS", bufs=12) as espool, \
             tc.tile_pool(name="attnsb", bufs=2) as attnsb, \
             tc.tile_pool(name="pss", bufs=2, space="PSUM") as pss, \
             tc.tile_pool(name="psb", bufs=1, space="PSUM") as psb, \
             tc.tile_pool(name="psav", bufs=2, space="PSUM") as psav:
            vsb = []

            def project_v():
                for t in range(LT):
                    if t % 2 == 0:
                        vt = vpool.tile([P, 2, H, DH + 2], FP8,
                                        tag=f"v{t // 2}", name=f"v{t // 2}")
                        vsb.append(vt)
                    vt = vsb[t // 2]
                    nc.vector.memset(vt[:, t % 2, :, DH:DH + 2], 1.0)
                    ps = psqkv.tile([P, NH], F32, tag="ps", name="ps_v")
                    for kk in range(KC // 2):
                        nc.tensor.matmul(
                            ps[:, :],
                            lhsT=h1T[:, 2 * kk:2 * kk + 2, t * P:(t + 1) * P],
                            rhs=wv1[:, 2 * kk:2 * kk + 2, :],
                            start=(kk == 0), stop=(kk == KC // 2 - 1),
                            perf_mode=DR)
                    nc.vector.tensor_scalar(
                        vt[:, t % 2, :, 0:DH],
                        ps.rearrange("p (h d) -> p h d", h=H),
                        DESC, None, op0=ALU.mult)

            pending = []
            for hp in range(KC):
                qT = proj_chunk(wq1, h1T, psqkv, hp, "qT", pool=qktp)
                kT = proj_chunk(wk1, h1T, psqkv, hp, "kT", pool=qktp)
                # scores + exp: both head-halves (sub) of one key tile share a
                # 2-bank PSUM tile and ONE exp instruction.
                es = {}
                for th in range(TH):
                    for lk in range(LT):
                        ps_s = pss.tile([P, 2 * NH], F32, tag="ps", name="ps_s")
                        for sub in (0, 1):
                            nc.tensor.matmul(
                                ps_s[:, sub * NH:(sub + 1) * NH],
                                lhsT=kT[sub * DH:(sub + 1) * DH,
                                        lk * P:(lk + 1) * P],
                                rhs=qT[sub * DH:(sub + 1) * DH,
                                       th * NH:(th + 1) * NH],
                                start=True, stop=True)
                        if lk % 2 == 0:
                            es[(th, lk // 2)] = espool.tile(
                                [P, 2, 2 * NH], FP8, tag="e", name="expS")
                        # -5 shift keeps e^(s-5) inside fp8e4m3 range; the
                        # softmax ratio cancels it exactly
                        nc.scalar.activation(es[(th, lk // 2)][:, lk % 2, :],
                                             ps_s[:, :], AF.Exp,
                                             bias=neg5[:, :])
                    if hp == 0 and th == 0:
                        # v projection overlaps the first exp stream
                        project_v()
                    for sub in (0, 1):
                        head = 2 * hp + sub
                        ps_o = psav.tile([P, NH], F32, tag="ps", name="ps_av")
                        for lkp in range(LT // 2):
                            nc.tensor.matmul(
                                ps_o[0:DH + 1, :],
                                lhsT=vsb[lkp][:, :, head, 0:DH + 1],
                                rhs=es[(th, lkp)][:, :,
                                                  sub * NH:(sub + 1) * NH],
                                start=(lkp == 0), stop=(lkp == LT // 2 - 1),
                                perf_mode=DR)
                        pend = (ps_o, o1T[sub * DH:(sub + 1) * DH, hp,
                                              th * NH:(th + 1) * NH])
                        pending.append(pend)
                        if len(pending) > 1:
                            po, od = pending.pop(0)
                            normalize(po, od, attnsb, psb)
            for po, od in pending:
                normalize(po, od, attnsb, psb)
        es_s1.close()

        # cross-attn weights: DMA now so they overlap attn1 tail
        es_a2 = ExitStack()
        a2pool = es_a2.enter_context(tc.tile_pool(name="a2w", bufs=1,
                                                  side="right"))
        wq2 = load_w_pk(a2pool, "a2_wq", "wq2pk")
        wo2 = load_w_pk(a2pool, "a2_wo", "wo2pk")
        wk2 = load_w(a2pool, "a2_wk", CD, dst_dtype=BF16, stage=wstage)
        wv2 = load_w(a2pool, "a2_wv", CD, dst_dtype=BF16, stage=wstage)

        # context K/V prep (independent of attn1) before the out-projection
        es_s2 = ExitStack()
        s2pool = es_s2.enter_context(tc.tile_pool(name="s2", bufs=1,
                                                  side="right"))
        ctxT, k2T = [], []
        v2 = s2pool.tile([P, H, DH + 1], BF16, tag="v2", name="v2")
        with tc.tile_pool(name="psctx", bufs=2, space="PSUM") as psctx:
            for cc in range(CC):
                ct = s2pool.tile([P, S], BF16, tag=f"ctxT{cc}", name=f"ctxT{cc}")
                ps = psctx.tile([P, P], F32, tag="ps", name="ps_ctxT")
                nc.tensor.transpose(ps[:, 0:S], ctx[0:S, cc * P:(cc + 1) * P],
                                    identity[0:S, 0:S])
                nc.scalar.copy(ct[:, :], ps[:, 0:S])
                ctxT.append(ct)
            for oc in range(KC):
                kt = s2pool.tile([P, S], BF16, tag=f"k2T{oc}", name=f"k2T{oc}")
                ps = psctx.tile([P, P], F32, tag="ps", name="ps_k2T")
                for cc in range(CC):
                    nc.tensor.matmul(ps[:, 0:S],
                                     lhsT=wk2[cc][:, oc * P:(oc + 1) * P],
                                     rhs=ctxT[cc][:, :],
                                     start=(cc == 0), stop=(cc == CC - 1))
                nc.scalar.copy(kt[:, :], ps[:, 0:S])
                k2T.append(kt)
            nc.vector.memset(v2[0:S, :, DH:DH + 1], 1.0)
            ps = psctx.tile([P, NH], F32, tag="psv", name="ps_v2")
            for cc in range(CC):
                nc.tensor.matmul(ps[0:S, :], lhsT=ctxT[cc][:, :],
                                 rhs=wv2[cc][:, :],
                                 start=(cc == 0), stop=(cc == CC - 1))
            nc.scalar.copy(v2[0:S, :, 0:DH],
                           ps[0:S, :].rearrange("p (h d) -> p h d", h=H))

        zs2 = [None] * LT
        proj_tok_residual(o1T, wo1, bo1_row, "1",
                          after_tile=lambda t: zs2.__setitem__(
                              t, ln_stats_tile(1, t)))
        es_a1.close()

        _sections = int(os.environ.get("BASS_KERNEL_SECTIONS", "3"))
        if _sections < 2:
            for t in range(LT):
                nc.sync.dma_start(out=out_p[t * P:(t + 1) * P, :],
                                  in_=xsb[t][:, :])
            es_s2.close()
            es_a2.close()
            es_ffw_skip = True
            return

        # ==================================================================
        # Section 2: cross-attention (keys/values from context, Lk = 77)
        # ==================================================================
        h2T = layer_norm_T(1, zs=zs2)

        # FF2 weights: DMA now (into space freed by a1w) to overlap attn2
        es_ffw = ExitStack()
        ffwpool = es_ffw.enter_context(tc.tile_pool(name="ffw", bufs=1))
        w2pk = []
        for jp in range(FC // 2):
            wt = ffwpool.tile([P, 2, D], FP8, tag=f"w2pk{jp}", name=f"w2pk{jp}")
            for h2 in range(2):
                r0 = (2 * jp + h2) * P
                wf = wstage.tile([P, D], F32, tag="stage", name=f"stg_w2_{jp}_{h2}")
                nc.sync.dma_start(out=wf[:, :], in_=pr["ff_w2"][r0:r0 + P, :])
                nc.gpsimd.tensor_copy(wt[:, h2, :], wf[:, :])
            w2pk.append(wt)

        o2T = opool.tile([P, KC, L], FP8, tag="o8", name="o2T8")
        with tc.tile_pool(name="psq2", bufs=1, space="PSUM") as psq2, \
             tc.tile_pool(name="qkt2", bufs=2) as qktp2, \
             tc.tile_pool(name="expS2", bufs=4) as es2pool, \
             tc.tile_pool(name="attnsb2", bufs=2) as attnsb2, \
             tc.tile_pool(name="pss2", bufs=2, space="PSUM") as pss2, \
             tc.tile_pool(name="psb2", bufs=1, space="PSUM") as psb2, \
             tc.tile_pool(name="psav2", bufs=2, space="PSUM") as psav2:
            pending = []
            for hp in range(KC):
                q2T = proj_chunk(wq2, h2T, psq2, hp, "q2T", pool=qktp2,
                                 evict="act")
                for th in range(TH):
                    ps_s = pss2.tile([P, 2 * NH], F32, tag="ps", name="ps_s2")
                    for sub in (0, 1):
                        nc.tensor.matmul(
                            ps_s[0:S, sub * NH:(sub + 1) * NH],
                            lhsT=k2T[hp][sub * DH:(sub + 1) * DH, :],
                            rhs=q2T[sub * DH:(sub + 1) * DH,
                                    th * NH:(th + 1) * NH],
                            start=True, stop=True)
                    e = es2pool.tile([P, 2 * NH], BF16, tag="e", name="expS2")
                    nc.scalar.activation(e[0:S, :], ps_s[0:S, :], AF.Exp)
                    for sub in (0, 1):
                        head = 2 * hp + sub
                        ps_o = psav2.tile([P, NH], F32, tag="ps", name="ps_av2")
                        nc.tensor.matmul(ps_o[0:DH + 1, :],
                                         lhsT=v2[0:S, head, :],
                                         rhs=e[0:S, sub * NH:(sub + 1) * NH],
                                         start=True, stop=True)
                        pend = (ps_o, o2T[sub * DH:(sub + 1) * DH, hp,
                                              th * NH:(th + 1) * NH])
                        pending.append(pend)
                        if len(pending) > 1:
                            po, od = pending.pop(0)
                            normalize(po, od, attnsb2, psb2, evict="act")
            for po, od in pending:
                normalize(po, od, attnsb2, psb2, evict="act")
        es_s2.close()

        zs3 = [None] * LT
        proj_tok_residual(o2T, wo2, bo2_row, "2",
                          after_tile=lambda t: zs3.__setitem__(
                              t, ln_stats_tile(2, t)))
        es_a2.close()

        # ==================================================================
        # Section 3: GEGLU feed-forward
        # ==================================================================
        if _sections < 3:
            for t in range(LT):
                nc.sync.dma_start(out=out_p[t * P:(t + 1) * P, :],
                                  in_=xsb[t][:, :])
            es_ffw.close()
            return

        # LN3 writes straight into the packed-fp8 [P, KC, L] moving operand.
        # Host pre-scales ff_w1 by 64 and ff_w2 by 64 so the fp8 weights stay
        # out of the e4m3 subnormal range; val rides a further 16x. The
        # evictions and the final residual undo the scales exactly.
        h3T = layer_norm_T(2, zs=zs3)

        es_s3 = ExitStack()
        prodpool = es_s3.enter_context(tc.tile_pool(name="prod", bufs=1))
        ffpiece = es_s3.enter_context(tc.tile_pool(name="ffpiece", bufs=2))

        # ff_w1 is read exactly once by PE: stream it as [128, KC, 512]
        # DoubleRow-packed fp8 groups (4 output chunks per group)
        def ff1_pieces(group, base, vg):
            pk = ffpiece.tile([P, KC, NH], FP8, tag=f"fp_{vg}",
                              name=f"ffw1_{vg}_{group}")
            for kc in range(KC):
                pf = wstage.tile([P, NH], F32, tag="stage",
                                 name=f"ffw1f_{vg}_{group}_{kc}")
                nc.sync.dma_start(
                    out=pf[:, :],
                    in_=pr["ff_w1"][kc * P:(kc + 1) * P,
                                    base + group * NH:base + (group + 1) * NH])
                nc.gpsimd.tensor_copy(pk[:, kc, :], pf[:, :])
            return pk

        prod8 = [prodpool.tile([P, 2, L], FP8, tag=f"prod{jp}",
                               name=f"prod{jp}") for jp in range(FC // 2)]
        DR = mybir.MatmulPerfMode.DoubleRow
        with tc.tile_pool(name="psff", bufs=2, space="PSUM") as psff, \
             tc.tile_pool(name="ffsb", bufs=3) as ffsb:
            nxt = (ff1_pieces(0, 0, "v"), ff1_pieces(0, 2 * FF // 2, "g"))
            for g in range(FC // 4):
              wv8, wg8 = nxt
              if g + 1 < FC // 4:
                  nxt = (ff1_pieces(g + 1, 0, "v"),
                         ff1_pieces(g + 1, 2 * FF // 2, "g"))
              for jj in range(4):
                j = g * 4 + jj
                # val and gate, each both token halves into a 2-bank PSUM tile
                ps_v = psff.tile([P, L], F32, tag="psv", name="ps_ffv")
                ps_g = psff.tile([P, L], F32, tag="psg", name="ps_ffg")
                # kk outer / th inner: consecutive matmuls share the
                # stationary weight slice, so its LDWEIGHTS is paid once
                for w8, ps_x in ((wv8, ps_v), (wg8, ps_g)):
                    for kk in range(KC // 2):
                        for th in range(TH):
                            nc.tensor.matmul(
                                ps_x[:, th * NH:(th + 1) * NH],
                                lhsT=w8[:, 2 * kk:2 * kk + 2,
                                        jj * P:(jj + 1) * P],
                                rhs=h3T[:, 2 * kk:2 * kk + 2,
                                        th * NH:(th + 1) * NH],
                                start=(kk == 0), stop=(kk == KC // 2 - 1),
                                perf_mode=DR)
                # val carries 16x (undone after ff2): the val half of ff_w1 is
                # host-scaled by 16 (not 64), so ps_v is already 16*val_noB;
                # one STT adds the (16x) bias and multiplies by gelu(gate).
                gel = ffsb.tile([P, L], BF16, tag="gel", name=f"gel{j}")
                nc.scalar.activation(gel[:, :], ps_g[:, :], AF.Gelu,
                                     bias=fb1[:, FC + j:FC + j + 1],
                                     scale=1.0 / 64.0)
                nc.vector.scalar_tensor_tensor(
                    prod8[j // 2][:, j % 2, :], ps_v[:, :], fb1[:, j:j + 1],
                    gel[:, :], op0=ALU.add, op1=ALU.mult)

        with tc.tile_pool(name="psff2", bufs=3, space="PSUM") as psff2:
            for t in range(LT):
                ps = psff2.tile([P, NH], F32, tag="ps", name="ps_ff2")
                nc.tensor.matmul(ps[:, :], lhsT=ones[0:1, 0:P],
                                 rhs=fb2_row[:, :], start=True, stop=False)
                for jp in range(FC // 2):
                    nc.tensor.matmul(ps[:, :],
                                     lhsT=prod8[jp][:, :, t * P:(t + 1) * P],
                                     rhs=w2pk[jp][:, :, :],
                                     start=False, stop=(jp == FC // 2 - 1),
                                     perf_mode=DR)
                # undo the 16*64 fp8 scaling (bias preload carries it too)
                nc.vector.scalar_tensor_tensor(
                    xsb[t][:, :], ps[:, :], 1.0 / 1024.0, xsb[t][:, :],
                    op0=ALU.mult, op1=ALU.add)
                nc.sync.dma_start(out=out_p[t * P:(t + 1) * P, :],
                                  in_=xsb[t][:, :])
        es_s3.close()
        es_ffw.close()


_NC_CACHE = {}


def _get_nc():
    if "nc" not in _NC_CACHE:
        _NC_CACHE["nc"] = _build_nc()
    return _NC_CACHE["nc"]


def prepare_in_maps(inputs):
    """Host-side preprocessing shared by kernel() and the bench harness:
    per-core input maps with the attention scale and fp8 range folds applied
    (DH^-0.5 into the query projections; x64 on all fp8-packed weights with
    matching bias scales, undone exactly on-device at the evictions)."""
    x = np.asarray(inputs["x"], dtype=np.float32)
    ctx = np.asarray(inputs["context"], dtype=np.float32)
    shared = {k: np.asarray(v, dtype=np.float32) for k, v in inputs.items()
              if k not in ("x", "context")}
    scale = np.float32(DH ** -0.5)
    w64 = np.float32(64.0)
    shared["a1_wq"] = np.ascontiguousarray(shared["a1_wq"] * (scale * w64))
    shared["a2_wq"] = np.ascontiguousarray(shared["a2_wq"] * (scale * w64))
    for nm in ("a1_wk", "a1_wv", "a1_wo", "a2_wo", "ff_w2"):
        shared[nm] = np.ascontiguousarray(shared[nm] * w64)
    # ff_w1: val half carries 16x (so PSUM holds 16*val directly, consumed by
    # the STT fused bias+product); gate half the usual 64x fp8 range fold
    w1 = np.array(shared["ff_w1"], dtype=np.float32)
    w1[:, :FF] *= np.float32(16.0)
    w1[:, FF:] *= w64
    shared["ff_w1"] = np.ascontiguousarray(w1)
    shared["a1_bo"] = np.ascontiguousarray(shared["a1_bo"] * w64)
    shared["a2_bo"] = np.ascontiguousarray(shared["a2_bo"] * w64)
    fb1s = np.array(shared["ff_b1"], dtype=np.float32)
    fb1s[:FF] *= np.float32(16.0)   # val carries 16x until after ff_w2
    shared["ff_b1"] = fb1s
    shared["ff_b2"] = np.ascontiguousarray(
        shared["ff_b2"] * np.float32(1024.0))
    in_maps = []
    for b in range(NCORES):
        m = {"x": np.ascontiguousarray(x[b]),
             "context": np.ascontiguousarray(ctx[b])}
        m.update(shared)
        in_maps.append(m)
    return in_maps


def kernel(**inputs):
    nc = _get_nc()
    in_maps = prepare_in_maps(inputs)
    res = run_bass_kernel_spmd(nc, in_maps, list(range(NCORES)))
    out = np.stack([res.results[i]["out"] for i in range(NCORES)], axis=0)
    return out.astype(np.float32)



# revision 15
# speedup vs baseline: 1.1620x; 1.1620x over previous
"""Trainium2 Bass kernel for a BasicTransformerBlock (self-attn + cross-attn + GEGLU FF).

Contract: kernel(**inputs) takes FULL unsharded inputs (np arrays keyed as in
setup_inputs()) and returns the FULL [8, 1024, 512] float32 output.

Sharding: data-parallel over batch B=8 across 8 NeuronCores (one batch element
per core, all weights replicated, no collectives).

Structure (per core, tokens L=1024, model D=512, heads H=8 x DH=64):
  - LayerNorm token-major (bn_stats), z in bf16, PE-transposed per channel
    chunk into one packed PSUM bank, single 2x-mode DVE eviction per chunk
    applying gamma/beta -> feature-major hT [128ch, 1024tok] bf16.
  - Projections feature-major: weights stationary (fp32r), hT moving (bf16).
  - Self-attention: per head-pair hp, scores S^T = K Q^T into a 2-bank PSUM
    tile (both 64-part head halves side by side), ONE exp -> bf16; AV rides
    the softmax denominator as V's extra ones-column; normalization via DVE
    reciprocal + GpSimd partition_broadcast + DVE multiply (no PE, no ACT).
  - Cross-attention analogous (Lk = 77).
  - GEGLU FF: ff_w1 streamed from HBM (read once), val+gate evictions both on
    ACT (bias fused), product on DVE in 4x mode (bf16, SBUF-only), ff2
    accumulated with rank-1 bias preload, residual add, DMA out.
  - attn scale DH^-0.5 is folded into a1_wq/a2_wq on the host.
"""

import os

import numpy as np

import concourse.bass as bass
import concourse.tile as tile
from concourse import mybir
from concourse.bass_utils import run_bass_kernel_spmd
from concourse.masks import make_identity

# ---------------------------------------------------------------------------
# Workaround: this toolchain's walrus encodes at most ONE sync-wait per
# instruction (2 for EventSemaphore). Tile attaches one wait per producer
# proc, so after scheduling we hoist excess waits onto prepended same-engine
# NOPs -- semantically identical (the engine blocks at the NOP instead).
# ---------------------------------------------------------------------------
def _legalize_wait_counts(nc, max_waits=1):
    n_moved = 0
    for f in nc.m.functions:
        for bb in f.blocks:
            out, changed = [], False
            for inst in bb.instructions:
                si = inst.sync_info
                waits = list(si.on_wait) if si is not None and si.on_wait else []
                cap = 2 if isinstance(inst, mybir.InstEventSemaphore) else max_waits
                if len(waits) > cap:
                    keep, rest = waits[:cap], waits[cap:]
                    for i in range(0, len(rest), max_waits):
                        out.append(mybir.InstNoOp(
                            name=f"{inst.name}-lw{i}",
                            engine=inst.engine,
                            bass_nofuse=True,
                            sync_info=mybir.SyncInfo(
                                on_wait=rest[i:i + max_waits], on_update=[]),
                        ))
                    si.on_wait = keep
                    inst.sync_info = si
                    n_moved += len(rest)
                    changed = True
                out.append(inst)
            if changed:
                bb.instructions = out
    return n_moved


# ---------------------------------------------------------------------------
# Problem shapes (hardcoded per contract)
# ---------------------------------------------------------------------------
P = 128
B, L, D, S, CD, H, DH = 8, 1024, 512, 77, 768, 8, 64
FF = 2048                 # GEGLU inner dim; ff_w1 is [D, 2*FF]
LT = L // P               # 8 token tiles
KC = D // P               # 4 channel chunks of the model dim
CC = CD // P              # 6 context channel chunks
NH = 512                  # moving-operand chunk (PSUM bank = 512 fp32)
TH = L // NH              # 2 token halves
FC = FF // P              # 16 chunks of the FF inner dim
EPS = 1e-5
NCORES = 8

F32 = mybir.dt.float32
F32R = mybir.dt.float32r
BF16 = mybir.dt.bfloat16
FP8 = mybir.dt.float8e4
AF = mybir.ActivationFunctionType
ALU = mybir.AluOpType


def _build_nc():
    nc = bass.Bass(target_bir_lowering=False, debug=(os.environ.get('BASS_KERNEL_DEBUG','0')=='1'))

    pr = {}
    pr["x"] = nc.declare_dram_parameter("x", [L, D], F32, isOutput=False)
    pr["context"] = nc.declare_dram_parameter("context", [S, CD], F32, isOutput=False)
    for nm in ("ln1_g", "ln1_b", "ln2_g", "ln2_b", "ln3_g", "ln3_b"):
        pr[nm] = nc.declare_dram_parameter(nm, [D], F32, isOutput=False)
    for nm in ("a1_bo", "a2_bo", "ff_b2"):
        # consumed (only) as fp32r matmul operands via the rank-1 bias trick
        pr[nm] = nc.declare_dram_parameter(nm, [D], F32R, isOutput=False)
    for nm in ("a1_wq", "a1_wk", "a1_wv", "a1_wo", "a2_wq", "a2_wo"):
        pr[nm] = nc.declare_dram_parameter(nm, [D, D], F32, isOutput=False)
    for nm in ("a2_wk", "a2_wv"):
        pr[nm] = nc.declare_dram_parameter(nm, [CD, D], F32, isOutput=False)
    pr["ff_w1"] = nc.declare_dram_parameter("ff_w1", [D, 2 * FF], F32, isOutput=False)
    pr["ff_b1"] = nc.declare_dram_parameter("ff_b1", [2 * FF], F32, isOutput=False)
    pr["ff_w2"] = nc.declare_dram_parameter("ff_w2", [FF, D], F32, isOutput=False)
    out_p = nc.declare_dram_parameter("out", [L, D], F32, isOutput=True)

    reps = int(os.environ.get("BASS_KERNEL_REPS", "1"))
    with tile.TileContext(nc) as tc, \
         nc.allow_low_precision(reason="bf16/fp32r tiles feed matmuls; all "
                                       "matmul accumulation stays fp32 in PSUM"):
        if reps > 1:
            with tc.For_i(0, reps, 1):
                _emit(nc, tc, pr, out_p)
        else:
            _emit(nc, tc, pr, out_p)
    if os.environ.get("BASS_KERNEL_SKIP_WAIT_LEGALIZE") != "1":
        _legalize_wait_counts(nc)
    return nc


def _emit(nc, tc, pr, out_p):
    from contextlib import ExitStack

    top = ExitStack()
    with top:
        # ------------------ persistent pools (whole kernel) -----------------
        const = top.enter_context(tc.tile_pool(name="const", bufs=1))
        xpool = top.enter_context(tc.tile_pool(name="xsb", bufs=1))
        htpool = top.enter_context(tc.tile_pool(name="hT", bufs=1))
        lntp = top.enter_context(tc.tile_pool(name="lnt", bufs=4))
        zpool = top.enter_context(tc.tile_pool(name="zp", bufs=1))
        qkpool = top.enter_context(tc.tile_pool(name="qkT", bufs=1))
        opool = top.enter_context(tc.tile_pool(name="oT", bufs=1))
        wstage = top.enter_context(tc.tile_pool(
            name="wstage",
            bufs=3 if os.environ.get("BASS_KERNEL_BIGDMA", "1") == "1" else 5))

        # ----------------------------- constants ---------------------------
        # x tiles first on the HWDGE queue; tiny constant DMAs go through the
        # GpSimd SWDGE queue so they don't serialize behind them
        lnstack = const.tile([P, P], F32, tag="lnstack", name="lnstack")
        for i, (gnm, bnm) in enumerate((("ln1_g", "ln1_b"), ("ln2_g", "ln2_b"),
                                        ("ln3_g", "ln3_b"))):
            nc.sync.dma_start(out=lnstack[i * 8:i * 8 + 4, :],
                                in_=pr[gnm].rearrange("(a f) -> a f", f=P))
            nc.sync.dma_start(out=lnstack[i * 8 + 4:i * 8 + 8, :],
                                in_=pr[bnm].rearrange("(a f) -> a f", f=P))
        fb1stack = const.tile([32, P], F32, tag="fb1stack", name="fb1stack")
        nc.sync.dma_start(out=fb1stack[:, :],
                            in_=pr["ff_b1"].rearrange("(a f) -> a f", f=P))
        # bias rows for the rank-1 PSUM-preload trick
        bo1_row = const.tile([1, D], F32R, tag="bo1row", name="bo1row")
        nc.sync.dma_start(out=bo1_row[:, :],
                            in_=pr["a1_bo"].rearrange("(o f) -> o f", o=1))
        bo2_row = const.tile([1, D], F32R, tag="bo2row", name="bo2row")
        nc.sync.dma_start(out=bo2_row[:, :],
                            in_=pr["a2_bo"].rearrange("(o f) -> o f", o=1))
        fb2_row = const.tile([1, D], F32R, tag="fb2row", name="fb2row")
        nc.sync.dma_start(out=fb2_row[:, :],
                            in_=pr["ff_b2"].rearrange("(o f) -> o f", o=1))

        identity = const.tile([P, P], F32, tag="identity", name="identity")
        make_identity(nc, identity[:, :])
        idbf = const.tile([P, P], BF16, tag="idbf", name="idbf")
        nc.gpsimd.tensor_copy(idbf[:, :], identity[:, :])
        ones_f = const.tile([1, P], F32, tag="ones_f", name="ones_f")
        nc.vector.memset(ones_f[:, :], 1.0)
        ones = const.tile([1, P], F32R, tag="ones", name="ones")
        nc.vector.tensor_copy(ones[:, :], ones_f[:, :])
        eps_t = const.tile([P, 1], F32, tag="eps", name="eps")
        nc.vector.memset(eps_t[:, :], EPS)
        neg5 = const.tile([P, 1], F32, tag="neg5", name="neg5")
        nc.vector.memset(neg5[:, :], -5.0)

        # ------------------------------ DMAs in -----------------------------
        xsb = []
        for t in range(LT):
            xt = xpool.tile([P, D], F32, tag=f"x{t}", name=f"x{t}")
            nc.sync.dma_start(out=xt[:, :], in_=pr["x"][t * P:(t + 1) * P, :])
            xsb.append(xt)
        ctx = xpool.tile([P, CD], F32, tag="ctx", name="ctx")
        nc.sync.dma_start(out=ctx[0:S, :], in_=pr["context"][:, :])

        BIGDMA = os.environ.get("BASS_KERNEL_BIGDMA", "1") == "1"

        def big_stage(name):
            # shared wide staging tile [P, 3072] f32 (12 KiB/partition)
            return wstage.tile([P, 6, D], F32, tag="stgw", name=name)

        def load_w_pk(pool, nm, tag):
            """[512,512] weight -> packed fp8 [P, KC, D] (DoubleRow layout)."""
            w = pool.tile([P, KC, D], FP8, tag=tag, name=tag)
            if BIGDMA:
                wf = big_stage(f"stg_{nm}")
                nc.sync.dma_start(
                    out=wf[:, 0:KC, :],
                    in_=pr[nm].rearrange("(c p) d -> p c d", p=P))
                nc.gpsimd.tensor_copy(w[:, :, :], wf[:, 0:KC, :])
            else:
                for c in range(KC):
                    wf = wstage.tile([P, D], F32, tag="stage", name=f"stg_{nm}{c}")
                    nc.sync.dma_start(out=wf[:, :], in_=pr[nm][c * P:(c + 1) * P, :])
                    nc.gpsimd.tensor_copy(w[:, c, :], wf[:, :])
            return w

        def load_w(pool, nm, rows, dst_dtype=F32, stage=None):
            """Load a [rows, 512] weight as row-chunk tiles; optional bf16
            cast staged through `stage` pool on the (otherwise idle) GpSimd."""
            nch = rows // P
            if BIGDMA and stage is not None:
                wf = big_stage(f"stg_{nm}")
                nc.sync.dma_start(
                    out=wf[:, 0:nch, :],
                    in_=pr[nm].rearrange("(c p) d -> p c d", p=P))
                w = pool.tile([P, nch, D], dst_dtype, tag=f"{nm}pk",
                              name=f"{nm}pk")
                nc.gpsimd.tensor_copy(w[:, :, :], wf[:, 0:nch, :])
                return [w[:, c, :] for c in range(nch)]
            tiles = []
            for c in range(nch):
                if stage is None:
                    w = pool.tile([P, D], dst_dtype, tag=f"{nm}{c}", name=f"{nm}{c}")
                    nc.sync.dma_start(out=w[:, :], in_=pr[nm][c * P:(c + 1) * P, :])
                else:
                    wf = stage.tile([P, D], F32, tag="stage", name=f"stg_{nm}{c}")
                    nc.sync.dma_start(out=wf[:, :], in_=pr[nm][c * P:(c + 1) * P, :])
                    w = pool.tile([P, D], dst_dtype, tag=f"{nm}{c}", name=f"{nm}{c}")
                    nc.gpsimd.tensor_copy(w[:, :], wf[:, :])
                tiles.append(w)
            return tiles

        es_a1 = ExitStack()
        a1pool = es_a1.enter_context(tc.tile_pool(name="a1w", bufs=1))
        wq1 = load_w_pk(a1pool, "a1_wq", "wq1pk")
        wk1 = load_w_pk(a1pool, "a1_wk", "wk1pk")
        wv1 = load_w_pk(a1pool, "a1_wv", "wv1pk")
        wo1 = load_w_pk(a1pool, "a1_wo", "wo1pk")

        # LN gamma/beta: six [512] vectors stacked as [4,128] rows -> [24,128],
        # one transpose -> gb [128,24]; ff_b1 [4096] -> [32,128] -> fb1 [128,32]
        gb = const.tile([P, 24], F32, tag="gb", name="gb")
        fb1 = const.tile([P, 32], F32, tag="fb1", name="fb1")
        with tc.tile_pool(name="psprep", bufs=2, space="PSUM") as psprep:
            pst = psprep.tile([P, P], F32, tag="pstr", name="pstr_gb")
            nc.tensor.transpose(pst[:, 0:24], lnstack[0:24, :], identity[0:24, 0:24])
            nc.vector.tensor_copy(gb[:, :], pst[:, 0:24])
            pst2 = psprep.tile([P, P], F32, tag="pstr", name="pstr_fb1")
            nc.tensor.transpose(pst2[:, 0:32], fb1stack[0:32, :], identity[0:32, 0:32])
            nc.vector.tensor_copy(fb1[:, :], pst2[:, 0:32])

        def g_col(i, kc):
            return gb[:, i * 8 + kc:i * 8 + kc + 1]

        def b_col(i, kc):
            return gb[:, i * 8 + 4 + kc:i * 8 + 4 + kc + 1]

        # ------------------------------------------------------------------
        # LayerNorm -> feature-major hT [128ch, 1024tok] bf16 x 4.
        # z in bf16; all 8 token-tile transposes of one channel chunk land in
        # ONE packed PSUM bank (bf16), evicted by a single 2x-mode DVE
        # tensor_scalar applying gamma/beta.
        # ------------------------------------------------------------------
        def ln_stats_tile(ln_i, t):
            """bn_stats -> rstd -> normalized z for one token tile."""
            st = lntp.tile([P, 6], F32, tag="bnstats", name="bnstats")
            nc.vector.bn_stats(st[:, :], xsb[t][:, :])
            mv = lntp.tile([P, 2], F32, tag="bnaggr", name="bnaggr")
            nc.vector.bn_aggr(mv[:, :], st[:, :])
            # rstd = exp(-0.5*ln(var+eps)); Ln/Exp share one ACT table
            # set with Identity/Copy (Sqrt would force a set switch).
            lnv = lntp.tile([P, 1], F32, tag="lnv", name="lnv")
            nc.scalar.activation(lnv[:, :], mv[:, 1:2], AF.Ln,
                                 bias=eps_t[:, :])
            rstd = lntp.tile([P, 1], F32, tag="rstd", name="rstd")
            nc.scalar.activation(rstd[:, :], lnv[:, :], AF.Exp, scale=-0.5)
            nmr = lntp.tile([P, 1], F32, tag="nmr", name="nmr")
            nc.vector.tensor_scalar(nmr[:, :], mv[:, 0:1], rstd[:, :], -1.0,
                                    op0=ALU.mult, op1=ALU.mult)
            z = zpool.tile([P, D], BF16, tag=f"z{t}", name=f"z{ln_i}_{t}")
            nc.scalar.activation(z[:, :], xsb[t][:, :], AF.Identity,
                                 bias=nmr[:, :], scale=rstd[:, :])
            return z

        def layer_norm_T(ln_i, zs=None):
            if zs is None:
                zs = [ln_stats_tile(ln_i, t) for t in range(LT)]
            h3 = htpool.tile([P, KC, L], FP8, tag="h3T8", name=f"hT8_{ln_i}")
            hT = [h3[:, c, :] for c in range(KC)]
            with tc.tile_pool(name=f"lnps{ln_i}", bufs=2, space="PSUM") as lnps:
                for c in range(KC):
                    ps = lnps.tile([P, L], BF16, tag="ps", name=f"lntr{c}")
                    for half in range(TH):
                        for t in range(half * LT // TH, (half + 1) * LT // TH):
                            nc.tensor.transpose(ps[:, t * P:(t + 1) * P],
                                                zs[t][:, c * P:(c + 1) * P],
                                                idbf[:, :])
                        sl = slice(half * NH, (half + 1) * NH)
                        # gamma/beta on ACT (idle here; DVE is the serial
                        # bottleneck in the LN windows)
                        nc.scalar.activation(hT[c][:, sl], ps[:, sl],
                                             AF.Identity,
                                             bias=b_col(ln_i, c),
                                             scale=g_col(ln_i, c))
            return h3

        # feature-major projection of one output chunk (both token halves into
        # a 2-bank PSUM tile, single eviction) -> [128, 1024] bf16
        DR = mybir.MatmulPerfMode.DoubleRow
        DESC = 1.0 / 64.0   # undo the host-side x64 fp8 range fold

        def proj_chunk(wpk, h8, pspool, oc, out_tag, pool=None, evict="dve"):
            pool = pool or qkpool
            ot = pool.tile([P, L], BF16, tag=out_tag, name=f"{out_tag}{oc}")
            for th in range(TH):
                ps = pspool.tile([P, NH], F32, tag="ps", name=f"ps_{out_tag}{oc}")
                for kk in range(KC // 2):
                    nc.tensor.matmul(
                        ps[:, :],
                        lhsT=wpk[:, 2 * kk:2 * kk + 2, oc * P:(oc + 1) * P],
                        rhs=h8[:, 2 * kk:2 * kk + 2, th * NH:(th + 1) * NH],
                        start=(kk == 0), stop=(kk == KC // 2 - 1),
                        perf_mode=DR)
                if evict == "act":
                    nc.scalar.mul(ot[:, th * NH:(th + 1) * NH], ps[:, :], DESC)
                else:
                    nc.vector.tensor_scalar(ot[:, th * NH:(th + 1) * NH],
                                            ps[:, :], DESC, None, op0=ALU.mult)
            return ot

        # out-projection + bias (rank-1 PSUM preload) + residual into xsb.
        # via_pool: stage PSUM->SBUF on ACT and add on GpSimd, freeing DVE in
        # the attention windows.
        def proj_tok_residual(o8, wpk, bias_row, nm, after_tile=None):
            with tc.tile_pool(name=f"psproj{nm}", bufs=3, space="PSUM") as psproj:
                for t in range(LT):
                    ps = psproj.tile([P, NH], F32, tag="ps", name="ps_proj")
                    nc.tensor.matmul(ps[:, :], lhsT=ones[0:1, 0:P],
                                     rhs=bias_row[:, :], start=True, stop=False)
                    for kk in range(KC // 2):
                        nc.tensor.matmul(
                            ps[:, :],
                            lhsT=o8[:, 2 * kk:2 * kk + 2, t * P:(t + 1) * P],
                            rhs=wpk[:, 2 * kk:2 * kk + 2, :],
                            start=False, stop=(kk == KC // 2 - 1),
                            perf_mode=DR)
                    nc.vector.scalar_tensor_tensor(
                        xsb[t][:, :], ps[:, :], DESC, xsb[t][:, :],
                        op0=ALU.mult, op1=ALU.add)
                    if after_tile is not None:
                        after_tile(t)

        # softmax normalization: denominator row (DH) of ps_o -> reciprocal
        # (DVE) -> rank-1 PE broadcast to 64 partitions -> evict -> multiply.
        # Emission is DEFERRED one unit by the callers so the PE ps_b matmul
        # never waits on the reciprocal.
        def normalize(ps_o, o_dst, attnsb, psb, evict="dve"):
            rec = attnsb.tile([1, NH], F32R, tag="rec", name="rec")
            nc.vector.reciprocal(rec[:, :], ps_o[DH:DH + 1, :])
            ps_b = psb.tile([P, NH], F32, tag="ps", name="ps_b")
            nc.tensor.matmul(ps_b[0:DH, :], lhsT=ones[0:1, 0:DH],
                             rhs=rec[:, :], start=True, stop=True)
            rb = attnsb.tile([DH, NH], F32, tag="rb", name="rb")
            if evict == "act":
                nc.scalar.copy(rb[:, :], ps_b[0:DH, :])
            else:
                nc.vector.tensor_copy(rb[:, :], ps_b[0:DH, :])
            nc.vector.tensor_mul(o_dst, ps_o[0:DH, :], rb[:, :])

        # ==================================================================
        # Section 1: self-attention
        # ==================================================================
        h1T = layer_norm_T(0)

        es_s1 = ExitStack()
        vpool = es_s1.enter_context(tc.tile_pool(name="vsb", bufs=1))
        o1T = opool.tile([P, KC, L], FP8, tag="o8", name="o1T8")

        with tc.tile_pool(name="psqkv", bufs=1, space="PSUM") as psqkv, \
             tc.tile_pool(name="qkt", bufs=2) as qktp, \
             tc.tile_pool(name="expS", bufs=12) as espool, \
             tc.tile_pool(name="attnsb", bufs=2) as attnsb, \
             tc.tile_pool(name="pss", bufs=2, space="PSUM") as pss, \
             tc.tile_pool(name="psb", bufs=1, space="PSUM") as psb, \
             tc.tile_pool(name="psav", bufs=2, space="PSUM") as psav:
            vsb = []

            def project_v():
                for t in range(LT):
                    if t % 2 == 0:
                        vt = vpool.tile([P, 2, H, DH + 2], FP8,
                                        tag=f"v{t // 2}", name=f"v{t // 2}")
                        vsb.append(vt)
                    vt = vsb[t // 2]
                    nc.vector.memset(vt[:, t % 2, :, DH:DH + 2], 1.0)
                    ps = psqkv.tile([P, NH], F32, tag="ps", name="ps_v")
                    for kk in range(KC // 2):
                        nc.tensor.matmul(
                            ps[:, :],
                            lhsT=h1T[:, 2 * kk:2 * kk + 2, t * P:(t + 1) * P],
                            rhs=wv1[:, 2 * kk:2 * kk + 2, :],
                            start=(kk == 0), stop=(kk == KC // 2 - 1),
                            perf_mode=DR)
                    nc.vector.tensor_scalar(
                        vt[:, t % 2, :, 0:DH],
                        ps.rearrange("p (h d) -> p h d", h=H),
                        DESC, None, op0=ALU.mult)

            pending = []
            for hp in range(KC):
                qT = proj_chunk(wq1, h1T, psqkv, hp, "qT", pool=qktp)
                kT = proj_chunk(wk1, h1T, psqkv, hp, "kT", pool=qktp)
                # scores + exp: both head-halves (sub) of one key tile share a
                # 2-bank PSUM tile and ONE exp instruction.
                es = {}
                for th in range(TH):
                    for lk in range(LT):
                        ps_s = pss.tile([P, 2 * NH], F32, tag="ps", name="ps_s")
                        for sub in (0, 1):
                            nc.tensor.matmul(
                                ps_s[:, sub * NH:(sub + 1) * NH],
                                lhsT=kT[sub * DH:(sub + 1) * DH,
                                        lk * P:(lk + 1) * P],
                                rhs=qT[sub * DH:(sub + 1) * DH,
                                       th * NH:(th + 1) * NH],
                                start=True, stop=True)
                        if lk % 2 == 0:
                            es[(th, lk // 2)] = espool.tile(
                                [P, 2, 2 * NH], FP8, tag="e", name="expS")
                        # -5 shift keeps e^(s-5) inside fp8e4m3 range; the
                        # softmax ratio cancels it exactly
                        nc.scalar.activation(es[(th, lk // 2)][:, lk % 2, :],
                                             ps_s[:, :], AF.Exp,
                                             bias=neg5[:, :])
                    if hp == 0 and th == 0:
                        # v projection overlaps the first exp stream
                        project_v()
                    for sub in (0, 1):
                        head = 2 * hp + sub
                        ps_o = psav.tile([P, NH], F32, tag="ps", name="ps_av")
                        for lkp in range(LT // 2):
                            nc.tensor.matmul(
                                ps_o[0:DH + 1, :],
                                lhsT=vsb[lkp][:, :, head, 0:DH + 1],
                                rhs=es[(th, lkp)][:, :,
                                                  sub * NH:(sub + 1) * NH],
                                start=(lkp == 0), stop=(lkp == LT // 2 - 1),
                                perf_mode=DR)
                        pend = (ps_o, o1T[sub * DH:(sub + 1) * DH, hp,
                                              th * NH:(th + 1) * NH])
                        pending.append(pend)
                        if len(pending) > 1:
                            po, od = pending.pop(0)
                            normalize(po, od, attnsb, psb)
            for po, od in pending:
                normalize(po, od, attnsb, psb)
        es_s1.close()

        # cross-attn weights: DMA now so they overlap attn1 tail
        es_a2 = ExitStack()
        a2pool = es_a2.enter_context(tc.tile_pool(name="a2w", bufs=1,
                                                  side="right"))
        wq2 = load_w_pk(a2pool, "a2_wq", "wq2pk")
        wo2 = load_w_pk(a2pool, "a2_wo", "wo2pk")
        wk2 = load_w(a2pool, "a2_wk", CD, dst_dtype=BF16, stage=wstage)
        wv2 = load_w(a2pool, "a2_wv", CD, dst_dtype=BF16, stage=wstage)

        # context K/V prep (independent of attn1) before the out-projection
        es_s2 = ExitStack()
        s2pool = es_s2.enter_context(tc.tile_pool(name="s2", bufs=1,
                                                  side="right"))
        ctxT, k2T = [], []
        v2 = s2pool.tile([P, H, DH + 1], BF16, tag="v2", name="v2")
        with tc.tile_pool(name="psctx", bufs=2, space="PSUM") as psctx:
            for cc in range(CC):
                ct = s2pool.tile([P, S], BF16, tag=f"ctxT{cc}", name=f"ctxT{cc}")
                ps = psctx.tile([P, P], F32, tag="ps", name="ps_ctxT")
                nc.tensor.transpose(ps[:, 0:S], ctx[0:S, cc * P:(cc + 1) * P],
                                    identity[0:S, 0:S])
                nc.scalar.copy(ct[:, :], ps[:, 0:S])
                ctxT.append(ct)
            for oc in range(KC):
                kt = s2pool.tile([P, S], BF16, tag=f"k2T{oc}", name=f"k2T{oc}")
                ps = psctx.tile([P, P], F32, tag="ps", name="ps_k2T")
                for cc in range(CC):
                    nc.tensor.matmul(ps[:, 0:S],
                                     lhsT=wk2[cc][:, oc * P:(oc + 1) * P],
                                     rhs=ctxT[cc][:, :],
                                     start=(cc == 0), stop=(cc == CC - 1))
                nc.scalar.copy(kt[:, :], ps[:, 0:S])
                k2T.append(kt)
            nc.vector.memset(v2[0:S, :, DH:DH + 1], 1.0)
            ps = psctx.tile([P, NH], F32, tag="psv", name="ps_v2")
            for cc in range(CC):
                nc.tensor.matmul(ps[0:S, :], lhsT=ctxT[cc][:, :],
                                 rhs=wv2[cc][:, :],
                                 start=(cc == 0), stop=(cc == CC - 1))
            nc.scalar.copy(v2[0:S, :, 0:DH],
                           ps[0:S, :].rearrange("p (h d) -> p h d", h=H))

        zs2 = [None] * LT
        proj_tok_residual(o1T, wo1, bo1_row, "1",
                          after_tile=lambda t: zs2.__setitem__(
                              t, ln_stats_tile(1, t)))
        es_a1.close()

        _sections = int(os.environ.get("BASS_KERNEL_SECTIONS", "3"))
        if _sections < 2:
            for t in range(LT):
                nc.sync.dma_start(out=out_p[t * P:(t + 1) * P, :],
                                  in_=xsb[t][:, :])
            es_s2.close()
            es_a2.close()
            es_ffw_skip = True
            return

        # ==================================================================
        # Section 2: cross-attention (keys/values from context, Lk = 77)
        # ==================================================================
        h2T = layer_norm_T(1, zs=zs2)

        # FF2 weights: DMA now (into space freed by a1w) to overlap attn2
        es_ffw = ExitStack()
        ffwpool = es_ffw.enter_context(tc.tile_pool(name="ffw", bufs=1))
        w2pk = []
        if BIGDMA:
            for b in range(FC // 4):
                wf = big_stage(f"stg_w2_{b}")
                nc.sync.dma_start(
                    out=wf[:, 0:4, :],
                    in_=pr["ff_w2"][b * 4 * P:(b + 1) * 4 * P, :]
                        .rearrange("(c p) d -> p c d", p=P))
                for q in range(2):
                    jp = 2 * b + q
                    wt = ffwpool.tile([P, 2, D], FP8, tag=f"w2pk{jp}",
                                      name=f"w2pk{jp}")
                    nc.gpsimd.tensor_copy(wt[:, :, :], wf[:, 2 * q:2 * q + 2, :])
                    w2pk.append(wt)
        else:
            for jp in range(FC // 2):
                wt = ffwpool.tile([P, 2, D], FP8, tag=f"w2pk{jp}", name=f"w2pk{jp}")
                for h2 in range(2):
                    r0 = (2 * jp + h2) * P
                    wf = wstage.tile([P, D], F32, tag="stage", name=f"stg_w2_{jp}_{h2}")
                    nc.sync.dma_start(out=wf[:, :], in_=pr["ff_w2"][r0:r0 + P, :])
                    nc.gpsimd.tensor_copy(wt[:, h2, :], wf[:, :])
                w2pk.append(wt)

        o2T = opool.tile([P, KC, L], FP8, tag="o8", name="o2T8")
        with tc.tile_pool(name="psq2", bufs=1, space="PSUM") as psq2, \
             tc.tile_pool(name="qkt2", bufs=2) as qktp2, \
             tc.tile_pool(name="expS2", bufs=4) as es2pool, \
             tc.tile_pool(name="attnsb2", bufs=2) as attnsb2, \
             tc.tile_pool(name="pss2", bufs=2, space="PSUM") as pss2, \
             tc.tile_pool(name="psb2", bufs=1, space="PSUM") as psb2, \
             tc.tile_pool(name="psav2", bufs=2, space="PSUM") as psav2:
            pending = []
            for hp in range(KC):
                q2T = proj_chunk(wq2, h2T, psq2, hp, "q2T", pool=qktp2,
                                 evict="act")
                for th in range(TH):
                    ps_s = pss2.tile([P, 2 * NH], F32, tag="ps", name="ps_s2")
                    for sub in (0, 1):
                        nc.tensor.matmul(
                            ps_s[0:S, sub * NH:(sub + 1) * NH],
                            lhsT=k2T[hp][sub * DH:(sub + 1) * DH, :],
                            rhs=q2T[sub * DH:(sub + 1) * DH,
                                    th * NH:(th + 1) * NH],
                            start=True, stop=True)
                    e = es2pool.tile([P, 2 * NH], BF16, tag="e", name="expS2")
                    nc.scalar.activation(e[0:S, :], ps_s[0:S, :], AF.Exp)
                    for sub in (0, 1):
                        head = 2 * hp + sub
                        ps_o = psav2.tile([P, NH], F32, tag="ps", name="ps_av2")
                        nc.tensor.matmul(ps_o[0:DH + 1, :],
                                         lhsT=v2[0:S, head, :],
                                         rhs=e[0:S, sub * NH:(sub + 1) * NH],
                                         start=True, stop=True)
                        pend = (ps_o, o2T[sub * DH:(sub + 1) * DH, hp,
                                              th * NH:(th + 1) * NH])
                        pending.append(pend)
                        if len(pending) > 1:
                            po, od = pending.pop(0)
                            normalize(po, od, attnsb2, psb2, evict="act")
            for po, od in pending:
                normalize(po, od, attnsb2, psb2, evict="act")
        es_s2.close()

        zs3 = [None] * LT
        proj_tok_residual(o2T, wo2, bo2_row, "2",
                          after_tile=lambda t: zs3.__setitem__(
                              t, ln_stats_tile(2, t)))
        es_a2.close()

        # ==================================================================
        # Section 3: GEGLU feed-forward
        # ==================================================================
        if _sections < 3:
            for t in range(LT):
                nc.sync.dma_start(out=out_p[t * P:(t + 1) * P, :],
                                  in_=xsb[t][:, :])
            es_ffw.close()
            return

        # LN3 writes straight into the packed-fp8 [P, KC, L] moving operand.
        # Host pre-scales ff_w1 by 64 and ff_w2 by 64 so the fp8 weights stay
        # out of the e4m3 subnormal range; val rides a further 16x. The
        # evictions and the final residual undo the scales exactly.
        h3T = layer_norm_T(2, zs=zs3)

        es_s3 = ExitStack()
        prodpool = es_s3.enter_context(tc.tile_pool(name="prod", bufs=1))
        ffpiece = es_s3.enter_context(tc.tile_pool(name="ffpiece", bufs=2))

        # ff_w1 is read exactly once by PE: stream it as [128, KC, 512]
        # DoubleRow-packed fp8 groups (4 output chunks per group)
        def ff1_pieces(group, base, vg):
            pk = ffpiece.tile([P, KC, NH], FP8, tag=f"fp_{vg}",
                              name=f"ffw1_{vg}_{group}")
            if BIGDMA:
                pf = big_stage(f"stg_ffw1_{vg}_{group}")
                nc.sync.dma_start(
                    out=pf[:, 0:KC, :],
                    in_=pr["ff_w1"][:, base + group * NH:base + (group + 1) * NH]
                        .rearrange("(c p) n -> p c n", p=P))
                nc.gpsimd.tensor_copy(pk[:, :, :], pf[:, 0:KC, :])
            else:
                for kc in range(KC):
                    pf = wstage.tile([P, NH], F32, tag="stage",
                                     name=f"ffw1f_{vg}_{group}_{kc}")
                    nc.sync.dma_start(
                        out=pf[:, :],
                        in_=pr["ff_w1"][kc * P:(kc + 1) * P,
                                        base + group * NH:base + (group + 1) * NH])
                    nc.gpsimd.tensor_copy(pk[:, kc, :], pf[:, :])
            return pk

        prod8 = [prodpool.tile([P, 2, L], FP8, tag=f"prod{jp}",
                               name=f"prod{jp}") for jp in range(FC // 2)]
        DR = mybir.MatmulPerfMode.DoubleRow
        with tc.tile_pool(name="psff", bufs=2, space="PSUM") as psff, \
             tc.tile_pool(name="ffsb", bufs=3) as ffsb:
            nxt = (ff1_pieces(0, 0, "v"), ff1_pieces(0, 2 * FF // 2, "g"))
            for g in range(FC // 4):
              wv8, wg8 = nxt
              if g + 1 < FC // 4:
                  nxt = (ff1_pieces(g + 1, 0, "v"),
                         ff1_pieces(g + 1, 2 * FF // 2, "g"))
              for jj in range(4):
                j = g * 4 + jj
                # val and gate, each both token halves into a 2-bank PSUM tile
                ps_v = psff.tile([P, L], F32, tag="psv", name="ps_ffv")
                ps_g = psff.tile([P, L], F32, tag="psg", name="ps_ffg")
                # kk outer / th inner: consecutive matmuls share the
                # stationary weight slice, so its LDWEIGHTS is paid once
                for w8, ps_x in ((wv8, ps_v), (wg8, ps_g)):
                    for kk in range(KC // 2):
                        for th in range(TH):
                            nc.tensor.matmul(
                                ps_x[:, th * NH:(th + 1) * NH],
                                lhsT=w8[:, 2 * kk:2 * kk + 2,
                                        jj * P:(jj + 1) * P],
                                rhs=h3T[:, 2 * kk:2 * kk + 2,
                                        th * NH:(th + 1) * NH],
                                start=(kk == 0), stop=(kk == KC // 2 - 1),
                                perf_mode=DR)
                # val carries 16x (undone after ff2): the val half of ff_w1 is
                # host-scaled by 16 (not 64), so ps_v is already 16*val_noB;
                # one STT adds the (16x) bias and multiplies by gelu(gate).
                gel = ffsb.tile([P, L], BF16, tag="gel", name=f"gel{j}")
                nc.scalar.activation(gel[:, :], ps_g[:, :], AF.Gelu,
                                     bias=fb1[:, FC + j:FC + j + 1],
                                     scale=1.0 / 64.0)
                if os.environ.get("BASS_KERNEL_FF_STT", "1") == "1":
                    nc.vector.scalar_tensor_tensor(
                        prod8[j // 2][:, j % 2, :], ps_v[:, :], fb1[:, j:j + 1],
                        gel[:, :], op0=ALU.add, op1=ALU.mult)
                else:
                    val = ffsb.tile([P, L], BF16, tag="val", name=f"val{j}")
                    nc.vector.tensor_scalar(val[:, :], ps_v[:, :], 1.0,
                                            fb1[:, j:j + 1],
                                            op0=ALU.mult, op1=ALU.add)
                    nc.vector.tensor_mul(prod8[j // 2][:, j % 2, :],
                                         val[:, :], gel[:, :])

        with tc.tile_pool(name="psff2", bufs=3, space="PSUM") as psff2:
            for t in range(LT):
                ps = psff2.tile([P, NH], F32, tag="ps", name="ps_ff2")
                nc.tensor.matmul(ps[:, :], lhsT=ones[0:1, 0:P],
                                 rhs=fb2_row[:, :], start=True, stop=False)
                for jp in range(FC // 2):
                    nc.tensor.matmul(ps[:, :],
                                     lhsT=prod8[jp][:, :, t * P:(t + 1) * P],
                                     rhs=w2pk[jp][:, :, :],
                                     start=False, stop=(jp == FC // 2 - 1),
                                     perf_mode=DR)
                # undo the 16*64 fp8 scaling (bias preload carries it too)
                nc.vector.scalar_tensor_tensor(
                    xsb[t][:, :], ps[:, :], 1.0 / 1024.0, xsb[t][:, :],
                    op0=ALU.mult, op1=ALU.add)
                nc.sync.dma_start(out=out_p[t * P:(t + 1) * P, :],
                                  in_=xsb[t][:, :])
        es_s3.close()
        es_ffw.close()


_NC_CACHE = {}


def _get_nc():
    if "nc" not in _NC_CACHE:
        _NC_CACHE["nc"] = _build_nc()
    return _NC_CACHE["nc"]


def prepare_in_maps(inputs):
    """Host-side preprocessing shared by kernel() and the bench harness:
    per-core input maps with the attention scale and fp8 range folds applied
    (DH^-0.5 into the query projections; x64 on all fp8-packed weights with
    matching bias scales, undone exactly on-device at the evictions)."""
    x = np.asarray(inputs["x"], dtype=np.float32)
    ctx = np.asarray(inputs["context"], dtype=np.float32)
    shared = {k: np.asarray(v, dtype=np.float32) for k, v in inputs.items()
              if k not in ("x", "context")}
    scale = np.float32(DH ** -0.5)
    w64 = np.float32(64.0)
    shared["a1_wq"] = np.ascontiguousarray(shared["a1_wq"] * (scale * w64))
    shared["a2_wq"] = np.ascontiguousarray(shared["a2_wq"] * (scale * w64))
    for nm in ("a1_wk", "a1_wv", "a1_wo", "a2_wo", "ff_w2"):
        shared[nm] = np.ascontiguousarray(shared[nm] * w64)
    # ff_w1: val half carries 16x (so PSUM holds 16*val directly, consumed by
    # the STT fused bias+product); gate half the usual 64x fp8 range fold
    w1 = np.array(shared["ff_w1"], dtype=np.float32)
    w1[:, :FF] *= np.float32(16.0)
    w1[:, FF:] *= w64
    shared["ff_w1"] = np.ascontiguousarray(w1)
    shared["a1_bo"] = np.ascontiguousarray(shared["a1_bo"] * w64)
    shared["a2_bo"] = np.ascontiguousarray(shared["a2_bo"] * w64)
    fb1s = np.array(shared["ff_b1"], dtype=np.float32)
    fb1s[:FF] *= np.float32(16.0)   # val carries 16x until after ff_w2
    shared["ff_b1"] = fb1s
    shared["ff_b2"] = np.ascontiguousarray(
        shared["ff_b2"] * np.float32(1024.0))
    in_maps = []
    for b in range(NCORES):
        m = {"x": np.ascontiguousarray(x[b]),
             "context": np.ascontiguousarray(ctx[b])}
        m.update(shared)
        in_maps.append(m)
    return in_maps


def kernel(**inputs):
    nc = _get_nc()
    in_maps = prepare_in_maps(inputs)
    res = run_bass_kernel_spmd(nc, in_maps, list(range(NCORES)))
    out = np.stack([res.results[i]["out"] for i in range(NCORES)], axis=0)
    return out.astype(np.float32)

